# revision 13
# baseline (speedup 1.0000x reference)
"""Contrastive (InfoNCE-style symmetric) loss on 8 trn2 NeuronCores.

Reference math (B=4096, D=1024, fp32):
    xn = x / max(||x_i||, eps);  yn = y / max(||y_j||, eps)
    S[i,j] = xn_i . yn_j ;  E = exp(S/tau)
    extra = B*eps + eps
    row_denom_i = sum_j E[i,j] + extra ; col_denom_j = sum_i E[i,j] + extra
    loss = -1/(2B) * ( 2*sum_i S_ii/tau - sum_i ln(row_denom_i)
                       - sum_j ln(col_denom_j) )

Sharding: batch dim of x is split across the 8 cores (512 rows each); every
core holds the full y (transposed, fp8). Each core computes its [4096, 512]
block of S^T on TensorE (j on partitions, local i on free) with fp8e4
DoubleRow matmuls (2 fp8 weights per PE cell, contraction 256 per matmul).

Structure:
  * No AllGather anywhere on the critical path: every core computes ALL 4096
    y norms locally from the fp8 y tiles (ones^T @ y^2, DoubleRow). The CC
    entry barrier is absorbed by a dependency-free dummy AllGather at t~0.
  * x is prescaled once by 8/||x_i|| (the 8 keeps fp8 x' out of the subnormal
    range) and quantized to fp8; the matmul then yields 8*rx_i*dot in PSUM and
    ACT applies exp with per-partition scale ry_j/(8*tau) reading PSUM
    directly - no per-block drain op.
  * 1/sqrt: exp(-0.5*ln(n) + bias) on ACT. Ln and Exp live in different ACT
    table sets on this stack, so the Lns/Exps are BATCHED (one [128,16] op
    per y-half) and a dummy Ln at t~0 prefetches the first table; only the
    rx Exp load (~1.3us) and the tail row-Ln load sit on the critical path.
  * Row-denominator matmuls (ones^T @ E-block, bf16) interleave 2 blocks
    behind the exp stream; column partials via DVE tensor_reduce on bf16 E.
    AllReduce 1 (cols 0-23) mid-stream; AllReduce 2a (cols 24-31) right after
    the last block; tiny AllReduce 2b (2 scalars) at the end overlaps the
    column-term postprocessing.
  * rx broadcast to [128,512] via a K=1 matmul (ones^T (x) rx) straight into
    PSUM - no DRAM round-trip.
  * y (fp8, 4MB) streams on the sync DMA queue while x/y_own (bf16, 2MB) use
    the vector queue in parallel.

Error budget: fp8e4m3 operands give S/tau errors ~0.02 absolute; across the
2*4096 log-sum terms this averages to ~1e-4 relative on the loss (gate 2e-2).
"""
import math

import numpy as np
import ml_dtypes

import concourse.bacc as bacc
import concourse.mybir as mybir
import concourse.tile as tile
from concourse.bass_utils import run_bass_kernel_spmd

AF = mybir.ActivationFunctionType
ALU = mybir.AluOpType
PM = mybir.MatmulPerfMode
BF16 = mybir.dt.bfloat16
FP8 = mybir.dt.float8e4
F32 = mybir.dt.float32

B = 4096
D = 1024
N_CORES = 8
BL = B // N_CORES          # 512 local x rows
TAU = 0.07
EPS = 1e-6
EXTRA = B * EPS + EPS
COEF = -1.0 / (2.0 * B)

ND = D // 128              # 8 contraction chunks of 128
NG = D // 256              # 4 DoubleRow groups of 256
NJB = B // 128             # 32 j-blocks
N_WARM = 12
ROW_LAG = 2

CS = 8.0                         # fp8 x' scale
RX_BIAS = math.log(CS)           # rx_scl = exp(-.5 ln nx2 + ln 8)  = 8/||x||
RY_BIAS = math.log(1.0 / (CS * TAU))  # ry_scl = 1/(8 tau ||y||)

_cache: dict = {}


def _build():
    nc = bacc.Bacc("TRN2", target_bir_lowering=False, debug=False,
                   num_devices=N_CORES)

    xT = nc.dram_tensor("xT", [D, BL], BF16, kind="ExternalInput")
    yT8 = nc.dram_tensor("yT8", [D, B], FP8, kind="ExternalInput")
    yTo = nc.dram_tensor("yTown", [D, BL], BF16, kind="ExternalInput")
    loss_out = nc.dram_tensor("loss", [1, 1], F32, kind="ExternalOutput")

    rg = [list(range(N_CORES))]

    with tile.TileContext(nc) as tc:
        with (
            tc.tile_pool(name="res", bufs=1) as res,
            tc.tile_pool(name="tmp", bufs=3) as tmp,
            tc.tile_pool(name="sqy", bufs=2) as sqp,
            tc.tile_pool(name="eblk", bufs=6) as epool,
            tc.tile_pool(name="pg", bufs=5, space="PSUM") as pg,
            tc.tile_pool(name="pa", bufs=2, space="PSUM") as pa,
            tc.tile_pool(name="prow", bufs=1, space="PSUM") as prow,
            tc.tile_pool(name="dram", bufs=1, space="DRAM") as dr,
        ):
            # ---- dummy collective: absorbs the one-time CC entry barrier ----
            dum_sb = res.tile([1, 8], F32, name="dum_sb")
            nc.gpsimd.memset(dum_sb[:], 1.0)
            dum_in = dr.tile([8], F32, name="dum_in")
            nc.gpsimd.dma_start(dum_in[:], dum_sb[:])
            dum_out = dr.tile([8 * N_CORES], F32, name="dum_out")
            nc.gpsimd.collective_compute(
                "AllGather", ALU.bypass, replica_groups=rg,
                ins=[dum_in.opt()], outs=[dum_out.opt()])

            # ---- PE warm-up ----
            wsrc = res.tile([128, 512], BF16, name="wsrc")
            nc.gpsimd.memset(wsrc[:], 0.125)
            wp = prow.tile([128, 512], F32, tag="prow", name="wp")
            for _ in range(N_WARM):
                nc.tensor.matmul(wp[:], wsrc[:, 0:128], wsrc[:],
                                 start=True, stop=True, skip_group_check=True)

            # ---- input DMAs: y fp8 on sync queue; x + y_own on vector ----
            # x: one [128, 8, 512] tile (d-chunk x local-i), one DMA
            xt8 = res.tile([128, ND, BL], BF16, name="xt8")
            nc.gpsimd.dma_start(
                xt8[:], xT[:].rearrange("(d p) i -> p d i", p=128))
            xts = [xt8[:, d, :] for d in range(ND)]
            # y fp8: per g2-half, two [128, 4, 2048] pieces on separate rings
            y8h = {}
            for g2 in range(2):
                t = res.tile([128, ND, 2048], FP8, tag=f"y8h{g2}",
                             name=f"y8h{g2}")
                src_ap = yT8[:, g2 * 2048:(g2 + 1) * 2048].rearrange(
                    "(d p) j -> p d j", p=128)
                if g2 == 0:
                    nc.gpsimd.dma_start(t[:, 0:4, :], src_ap[:, 0:4, :])
                    nc.scalar.dma_start(t[:, 4:8, :], src_ap[:, 4:8, :])
                else:
                    nc.sync.dma_start(t[:, 0:4, :], src_ap[:, 0:4, :])
                    nc.sync.dma_start(t[:, 4:8, :], src_ap[:, 4:8, :])
                y8h[g2] = t
            # DoubleRow weight views: [128, 2, 2048] per (g2, pair-group)
            y8 = {(g2, g): y8h[g2][:, 2 * g:2 * g + 2, :]
                  for g2 in range(2) for g in range(NG)}
            # y own shard: one [128, 8, 512] tile, one DMA
            yto8 = res.tile([128, ND, BL], BF16, name="yto8")
            nc.scalar.dma_start(
                yto8[:], yTo[:].rearrange("(d p) i -> p d i", p=128))
            ytos = [yto8[:, d, :] for d in range(ND)]

            ones_bf = res.tile([128, 1], BF16, name="ones_bf")
            nc.gpsimd.memset(ones_bf[:], 1.0)
            ones_f = res.tile([128, 1], F32, name="ones_f")
            nc.gpsimd.memset(ones_f[:], 1.0)
            ones1f = res.tile([1, 128], F32, name="ones1f")
            nc.gpsimd.memset(ones1f[:], 1.0)
            ones8 = res.tile([128, 2, 16], FP8, name="ones8")
            nc.gpsimd.memset(ones8[:], 1.0)
            b_ry = res.tile([128, 1], F32, name="b_ry")
            nc.gpsimd.memset(b_ry[:], RY_BIAS)
            b_rx = res.tile([1, 1], F32, name="b_rx")
            nc.gpsimd.memset(b_rx[:], RX_BIAS)

            # ---- ACT: prefetch the Ln table set with a dummy ----
            pre_in = res.tile([1, 1], F32, name="pre_in")
            nc.gpsimd.memset(pre_in[:], 1.0)
            pre_out = res.tile([1, 1], F32, name="pre_out")
            nc.scalar.activation(pre_out[:], pre_in[:], AF.Ln)

            # ---- rx chain: ||x||^2 -> 8/||x|| -> PE broadcast -> x' fp8 ----
            p_nx = pa.tile([1, 512], F32, tag="pa", name="p_nx")
            for d in range(ND):
                sq = tmp.tile([128, 512], BF16, tag="sq", name="sq")
                nc.vector.tensor_mul(sq[:], xts[d][:], xts[d][:])
                nc.tensor.matmul(p_nx[:], ones_bf[:], sq[:],
                                 start=(d == 0), stop=(d == ND - 1))
            lnx = tmp.tile([1, 512], F32, tag="v", name="lnx")
            nc.scalar.activation(lnx[:], p_nx[:], AF.Ln)
            rx = res.tile([1, 512], F32, name="rx")
            nc.scalar.activation(rx[:], lnx[:], AF.Exp, scale=-0.5,
                                 bias=b_rx[:, :])
            pb_rx = pa.tile([128, 512], F32, tag="pa", name="pb_rx")
            nc.tensor.matmul(pb_rx[:], ones1f[:], rx[:],
                             start=True, stop=True, skip_group_check=True)
            x8 = []
            for g in range(NG):
                t = res.tile([128, 2, 512], FP8, tag=f"x8_{g}", name=f"x8_{g}")
                x8.append(t)
            for g in range(NG):
                for h in range(2):
                    nc.vector.tensor_mul(x8[g][:, h, :], xts[2 * g + h][:],
                                         pb_rx[:])

            # ---- all-y norms, one g2-half at a time; batched Ln/Exp ----
            ny2_d = dr.tile([B], F32, name="ny2_d")
            ny2_p = res.tile([128, 32], F32, name="ny2_p")
            ry_scl = res.tile([128, 32], F32, name="ry_scl")
            ryo_scl = res.tile([1, 512], F32, name="ryo_scl")

            def emit_ynorm_half(g2):
                # squares per jc-chunk, split DVE (g 0-1) / ACT (g 2-3)
                sq8 = sqp.tile([128, ND, 2048], FP8, tag="sqy",
                               name=f"sq8_{g2}")
                yh = y8h[g2]
                for jc in range(4):
                    js = slice(jc * 512, (jc + 1) * 512)
                    nc.vector.tensor_mul(sq8[:, 0:4, js], yh[:, 0:4, js],
                                         yh[:, 0:4, js])
                    nc.scalar.activation(sq8[:, 4:8, js], yh[:, 4:8, js],
                                         AF.Square)
                    c = g2 * 4 + jc
                    p_ny = pa.tile([16, 512], F32, tag="pa", name=f"p_ny{c}")
                    for g in range(NG):
                        nc.tensor.matmul(
                            p_ny[:], ones8[:],
                            sq8[:, 2 * g:2 * g + 2, js],
                            start=(g == 0), stop=(g == NG - 1),
                            perf_mode=PM.DoubleRow, skip_group_check=True)
                    ny2c = tmp.tile([1, 512], F32, tag="v", name=f"ny2c{c}")
                    nc.vector.tensor_copy(ny2c[:], p_ny[0:1, :])
                    nc.gpsimd.dma_start(ny2_d[c * 512:(c + 1) * 512], ny2c[:])
                    nc.gpsimd.dma_start(
                        ny2_p[:, 4 * c:4 * c + 4],
                        ny2_d[c * 512:(c + 1) * 512].rearrange(
                            "(a b) -> b a", b=128))
                # one Ln + one Exp for the whole half
                lny = tmp.tile([128, 16], F32, tag="l16", name=f"lny{g2}")
                nc.scalar.activation(lny[:], ny2_p[:, 16 * g2:16 * (g2 + 1)],
                                     AF.Ln)
                nc.scalar.activation(ry_scl[:, 16 * g2:16 * (g2 + 1)], lny[:],
                                     AF.Exp, scale=-0.5, bias=b_ry[:, :])

            emit_ynorm_half(0)

            # ---- main stream ----
            colpart = res.tile([128, 32], F32, name="colpart")
            dk_rk = res.tile([1, 8], F32, name="dk_rk")
            nc.vector.memset(dk_rk[:], 0.0)
            e_blks = {}
            p_row = prow.tile([1, 512], F32, tag="prow", name="p_row")
            ar1_in = dr.tile([3072], F32, name="ar1_in")
            ar1_out = dr.tile([3072], F32, name="ar1_out")
            ar2_in = dr.tile([1032], F32, name="ar2_in")
            ar2_out = dr.tile([1032], F32, name="ar2_out")

            def emit_rowmm(jb):
                nc.tensor.matmul(p_row[:], ones_bf[:], e_blks.pop(jb)[:],
                                 start=(jb == 0), stop=(jb == NJB - 1),
                                 skip_group_check=True)

            def emit_ownnorm():
                p_no = pa.tile([1, 512], F32, tag="pa", name="p_no")
                for d in range(ND):
                    sqo = tmp.tile([128, 512], BF16, tag="sq", name="sqo")
                    nc.vector.tensor_mul(sqo[:], ytos[d][:], ytos[d][:])
                    nc.tensor.matmul(p_no[:], ones_bf[:], sqo[:],
                                     start=(d == 0), stop=(d == ND - 1),
                                     skip_group_check=True)
                lno = tmp.tile([1, 512], F32, tag="v", name="lno")
                nc.scalar.activation(lno[:], p_no[:], AF.Ln)
                nc.scalar.activation(ryo_scl[:], lno[:], AF.Exp, scale=-0.5,
                                     bias=b_ry[0:1, :])

            def emit_diag():
                p_dd = pa.tile([1, 512], F32, tag="pa", name="p_dd")
                for d in range(ND):
                    prd = tmp.tile([128, 512], BF16, tag="sq", name="prd")
                    g, h = d // 2, d % 2
                    nc.vector.tensor_mul(prd[:], x8[g][:, h, :], ytos[d][:])
                    nc.tensor.matmul(p_dd[:], ones_bf[:], prd[:],
                                     start=(d == 0), stop=(d == ND - 1),
                                     skip_group_check=True)
                v2 = tmp.tile([1, 512], F32, tag="v", name="v2")
                nc.vector.tensor_mul(v2[:], p_dd[:], ryo_scl[:])
                nc.vector.tensor_reduce(dk_rk[:, 0:1], v2[:],
                                        op=ALU.add, axis=mybir.AxisListType.X)

            for jb in range(NJB):
                g2, joff = jb // 16, (jb % 16) * 128
                pgt = pg.tile([128, 512], F32, tag="pg", name="pg")
                for g in range(NG):
                    nc.tensor.matmul(
                        pgt[:],
                        y8h[g2][:, 2 * g:2 * g + 2, joff:joff + 128],
                        x8[g][:],
                        start=(g == 0), stop=(g == NG - 1),
                        perf_mode=PM.DoubleRow, skip_group_check=True)
                eb = epool.tile([128, 512], BF16, tag="eb", name="eb")
                nc.scalar.activation(eb[:], pgt[:], AF.Exp,
                                     scale=ry_scl[:, jb:jb + 1])
                nc.vector.tensor_reduce(colpart[:, jb:jb + 1], eb[:],
                                        op=ALU.add, axis=mybir.AxisListType.X)
                e_blks[jb] = eb

                if jb >= ROW_LAG:
                    emit_rowmm(jb - ROW_LAG)
                if jb == 4:
                    emit_ynorm_half(1)
                if jb == 8:
                    emit_ownnorm()
                if jb == 10:
                    emit_diag()
                if jb == 23:
                    nc.gpsimd.dma_start(ar1_in[:], colpart[:, 0:24])
                    nc.gpsimd.collective_compute(
                        "AllReduce", ALU.add, replica_groups=rg,
                        ins=[ar1_in.opt()], outs=[ar1_out.opt()])

            for jb in range(NJB - ROW_LAG, NJB):
                emit_rowmm(jb)

            # ---- row term ----
            rdv = tmp.tile([1, 512], F32, tag="v", name="rdv")
            nc.vector.tensor_scalar_add(rdv[:], p_row[:], EXTRA)
            rlnv = tmp.tile([1, 512], F32, tag="v", name="rlnv")
            nc.scalar.activation(rlnv[:], rdv[:], AF.Ln,
                                 accum_out=dk_rk[:, 1:2])

            # ---- AllReduce 2: cols 24-31 + the two scalars ----
            nc.gpsimd.dma_start(ar2_in[0:1024], colpart[:, 24:32])
            nc.gpsimd.dma_start(ar2_in[1024:1032], dk_rk[:])
            nc.gpsimd.collective_compute(
                "AllReduce", ALU.add, replica_groups=rg,
                ins=[ar2_in.opt()], outs=[ar2_out.opt()])

            # ---- col terms (overlap the AR latencies) ----
            csum1 = tmp.tile([128, 24], F32, tag="w", name="csum1")
            nc.gpsimd.dma_start(csum1[:], ar1_out[:])
            cd1 = tmp.tile([128, 24], F32, tag="w", name="cd1")
            nc.vector.tensor_scalar_add(cd1[:], csum1[:], EXTRA)
            cln1 = tmp.tile([128, 24], F32, tag="w", name="cln1")
            cacc = res.tile([128, 2], F32, name="cacc")
            nc.scalar.activation(cln1[:], cd1[:], AF.Ln,
                                 accum_out=cacc[:, 0:1])
            csum2 = tmp.tile([128, 8], F32, tag="w2", name="csum2")
            nc.gpsimd.dma_start(csum2[:], ar2_out[0:1024])
            cd2 = tmp.tile([128, 8], F32, tag="w2", name="cd2")
            nc.vector.tensor_scalar_add(cd2[:], csum2[:], EXTRA)
            cln2 = tmp.tile([128, 8], F32, tag="w2", name="cln2")
            nc.scalar.activation(cln2[:], cd2[:], AF.Ln,
                                 accum_out=cacc[:, 1:2])
            p_s = pa.tile([1, 1], F32, tag="pa", name="p_s")
            nc.tensor.matmul(p_s[:], ones_f[:], cacc[:, 0:1],
                             start=True, stop=False, skip_group_check=True)
            nc.tensor.matmul(p_s[:], ones_f[:], cacc[:, 1:2],
                             start=False, stop=True, skip_group_check=True)

            sc2 = tmp.tile([1, 2], F32, tag="s2", name="sc2", bufs=1)
            nc.gpsimd.dma_start(sc2[:], ar2_out[1024:1026])
            f1 = res.tile([1, 1], F32, name="f1")
            nc.vector.tensor_scalar_mul(f1[:], sc2[:, 0:1], 2.0)
            f2 = res.tile([1, 1], F32, name="f2")
            nc.vector.tensor_sub(f2[:], f1[:], sc2[:, 1:2])
            f3 = res.tile([1, 1], F32, name="f3")
            nc.vector.tensor_sub(f3[:], f2[:], p_s[:])
            fl = res.tile([1, 1], F32, name="fl")
            nc.vector.tensor_scalar_mul(fl[:], f3[:], COEF)
            nc.sync.dma_start(loss_out[:, :], fl[:])

    nc.compile()
    return nc


def get_nc():
    if "nc" not in _cache:
        _cache["nc"] = _build()
    return _cache["nc"]


def make_in_maps(x: np.ndarray, y: np.ndarray):
    xb = x.astype(ml_dtypes.bfloat16)
    y8 = np.clip(y, -240.0, 240.0).astype(ml_dtypes.float8_e4m3)
    yb = y.astype(ml_dtypes.bfloat16)
    xT = np.ascontiguousarray(xb.T)
    yT8 = np.ascontiguousarray(y8.T)
    yT = np.ascontiguousarray(yb.T)
    in_maps = []
    for k in range(N_CORES):
        in_maps.append({
            "xT": np.ascontiguousarray(xT[:, k * BL:(k + 1) * BL]),
            "yT8": yT8,
            "yTown": np.ascontiguousarray(yT[:, k * BL:(k + 1) * BL]),
        })
    return in_maps


def kernel(x: np.ndarray, y: np.ndarray) -> np.ndarray:
    nc = get_nc()
    in_maps = make_in_maps(np.asarray(x), np.asarray(y))
    res = run_bass_kernel_spmd(nc, in_maps, core_ids=list(range(N_CORES)))
    loss = res.results[0]["loss"]
    return np.asarray(loss, dtype=np.float32).reshape(())
